# revision 1
# baseline (speedup 1.0000x reference)
"""KoLeo-loss kernel for Trainium2 (Bass/Tile), data-parallel over batch on 8 cores.

Input : student_output [8, 4096, 256] fp32
Output: scalar fp32 loss = -mean(log(||x - x_nn + 1e-8||_2 + 1e-8))
        where x_nn[b,t] = x[b, argmax_s <x[b,t], x[b,s]> (diag excluded)].

Per-core plan (core b handles batch b):
  - PE: gram matrix dots = x @ x.T in 32 m-tiles of [128, 4096]
        (2 K-chunks of 128 x 8 N-blocks of 512, fp32 PSUM accumulation)
  - ACT: PSUM -> SBUF copies
  - DVE: per-row top-8 values (nc.vector.max) + their indices
        (nc.vector.max_index).  The diagonal (self inner product) is the
        row max with overwhelming probability; drop it by value-matching
        the top-1 index against the diagonal column id and falling back
        to the top-2 index.
  - GPSIMD indirect DMA: gather neighbor rows x[I[t]] from HBM
  - DVE/ACT: dist2[t] = sum_d (x[t,d] - x_nn[t,d] + 1e-8)^2
  - host: loss = -mean(log(sqrt(dist2) + 1e-8)) in f64, over all 8 cores.
"""

import numpy as np

import concourse.bass as bass
import concourse.tile as tile
from concourse import bacc, mybir
from concourse import bass_utils

F32 = mybir.dt.float32
U32 = mybir.dt.uint32

B, T, D = 8, 4096, 256
P = 128                  # partitions
M = T // P               # 32 m-tiles
KC = D // P              # 2 contraction chunks
NB = T // 512            # 8 n-blocks of 512
EPS = 1e-8


def build_bass(num_devices=8):
    nc = bacc.Bacc("TRN2", target_bir_lowering=False, debug=False,
                   num_devices=num_devices)
    xT = nc.dram_tensor("xT", [KC, P, T], F32, kind="ExternalInput")
    xr = nc.dram_tensor("xr", [P, M * D], F32, kind="ExternalInput")
    xg = nc.dram_tensor("xg", [T, D], F32, kind="ExternalInput")
    d2_out = nc.dram_tensor("d2", [P, M], F32, kind="ExternalOutput")

    with tile.TileContext(nc) as tc:
        with (
            tc.tile_pool(name="const", bufs=1) as const_pool,
            tc.tile_pool(name="dots", bufs=2) as dots_pool,
            tc.tile_pool(name="psum", bufs=2, space="PSUM") as psum_pool,
            tc.tile_pool(name="small", bufs=4) as small_pool,
            tc.tile_pool(name="res", bufs=1) as res_pool,
        ):
            # resident inputs
            xT_sb = [const_pool.tile([P, T], F32, name=f"xT{c}", tag=f"xT{c}") for c in range(KC)]
            for c in range(KC):
                nc.sync.dma_start(xT_sb[c][:], xT[c])
            xr_sb = const_pool.tile([P, M * D], F32, tag="xr")
            nc.sync.dma_start(xr_sb[:], xr[:])

            # diag column ids: diagcol[p, m] = 128*m + p (exact in fp32)
            diagcol = const_pool.tile([P, M], F32, tag="diagcol")
            nc.gpsimd.iota(diagcol[:], pattern=[[P, M]], base=0,
                           channel_multiplier=1,
                           allow_small_or_imprecise_dtypes=True)

            epsb = const_pool.tile([P, 1], F32, tag="epsb")
            nc.vector.memset(epsb[:], EPS)
            d2_all = res_pool.tile([P, M], F32, tag="d2")
            icol_all = res_pool.tile([P, M], U32, tag="icol")

            xnn_tiles = [None] * M

            def finish(m):
                # dist2 for m-tile m (issued 2 iterations later so the
                # gather has long completed; keeps ACT/DVE streams stall-free)
                xnn = xnn_tiles[m]
                diff = small_pool.tile([P, D], F32, tag="diff")
                nc.vector.tensor_tensor(
                    out=diff[:], in0=xr_sb[:, m * D:(m + 1) * D], in1=xnn[:],
                    op=mybir.AluOpType.subtract)
                sq = small_pool.tile([P, D], F32, tag="sq")
                nc.scalar.activation(
                    out=sq[:], in_=diff[:],
                    func=mybir.ActivationFunctionType.Square,
                    bias=epsb[:], scale=1.0,
                    accum_out=d2_all[:, m:m + 1])

            for m in range(M):
                dots = dots_pool.tile([P, T], F32, tag="dots")
                for h in range(2):          # two psum halves of 4 n-blocks
                    ps = psum_pool.tile([P, 2048], F32, tag="ps")
                    for jj in range(4):
                        j = 4 * h + jj
                        for c in range(KC):
                            nc.tensor.matmul(
                                ps[:, jj * 512:(jj + 1) * 512],
                                lhsT=xT_sb[c][:, m * P:(m + 1) * P],
                                rhs=xT_sb[c][:, j * 512:(j + 1) * 512],
                                start=(c == 0), stop=(c == KC - 1))
                    for jj in range(4):
                        j = 4 * h + jj
                        nc.scalar.copy(dots[:, j * 512:(j + 1) * 512],
                                       ps[:, jj * 512:(jj + 1) * 512])

                top8 = small_pool.tile([P, 8], F32, tag="top8")
                nc.vector.max(out=top8[:], in_=dots[:])
                idx8 = small_pool.tile([P, 8], U32, tag="idx8")
                nc.vector.max_index(out=idx8[:], in_max=top8[:], in_values=dots[:])

                # neighbor index: idx1 unless idx1 is the diagonal -> idx2
                idx1f = small_pool.tile([P, 1], F32, tag="idx1f")
                nc.vector.tensor_copy(idx1f[:], idx8[:, 0:1])
                mask = small_pool.tile([P, 1], U32, tag="mask")
                nc.vector.tensor_scalar(
                    out=mask[:], in0=idx1f[:], scalar1=diagcol[:, m:m + 1],
                    scalar2=None, op0=mybir.AluOpType.is_equal)
                nc.vector.select(icol_all[:, m:m + 1], mask[:],
                                 idx8[:, 1:2], idx8[:, 0:1])

                # gather x[I[t], :] rows from HBM
                xnn = small_pool.tile([P, D], F32, tag="xnn")
                xnn_tiles[m] = xnn
                nc.gpsimd.indirect_dma_start(
                    out=xnn[:], out_offset=None,
                    in_=xg[:],
                    in_offset=bass.IndirectOffsetOnAxis(
                        ap=icol_all[:, m:m + 1], axis=0))

                if m >= 2:
                    finish(m - 2)
            finish(M - 2)
            finish(M - 1)

            nc.sync.dma_start(d2_out[:], d2_all[:])
    nc.compile()
    return nc


_CACHE = {}


def _built():
    if "nc" not in _CACHE:
        _CACHE["nc"] = build_bass(8)
    return _CACHE["nc"]


def make_in_maps(x):
    x = np.ascontiguousarray(np.asarray(x, dtype=np.float32))
    assert x.shape == (B, T, D)
    in_maps = []
    for b in range(B):
        xb = x[b]
        in_maps.append({
            "xT": np.ascontiguousarray(xb.T).reshape(KC, P, T),
            "xr": np.ascontiguousarray(
                xb.reshape(M, P, D).transpose(1, 0, 2)).reshape(P, M * D),
            "xg": xb,
        })
    return in_maps


def postprocess(d2_list):
    # d2_list: per-core [128, 32] fp32 squared distances (row t = 128*m + p)
    total = 0.0
    n = 0
    for d2 in d2_list:
        d = np.sqrt(d2.astype(np.float64))
        total += np.log(d + EPS).sum()
        n += d.size
    return np.float32(-(total / n))


def kernel(student_output):
    nc = _built()
    in_maps = make_in_maps(student_output)
    res = bass_utils.run_bass_kernel_spmd(nc, in_maps, core_ids=list(range(B)))
    return postprocess([res.results[b]["d2"] for b in range(B)])



# revision 6
# speedup vs baseline: 8079.5097x; 8079.5097x over previous
"""KoLeo-loss kernel for Trainium2 (Bass/Tile), data-parallel over batch on 8 cores.

Input : student_output [8, 4096, 256] fp32
Output: scalar fp32 loss = -mean(log(||x - x_nn + 1e-8||_2 + 1e-8))
        where x_nn[b,t] = x[b, argmax_s <x[b,t], x[b,s]> (diag excluded)].

Per-core plan (core b handles batch b):
  - PE: gram matrix dots = x @ x.T in 32 m-tiles of [128, 4096], bf16
        inputs (fp32 PSUM accumulation; bf16 matmul streams 1 col/cycle
        vs 4 for fp32).
  - ACT: PSUM -> SBUF fp32 copies (frees PSUM for the next half-tile).
  - DVE: InstMax top-8 + InstMaxIndex over each staged [128, 4096] row.
        The diagonal self-dot (~256) is always the row max (off-diag
        maxes are ~65 for this data), so column 1 of the top-8 is the
        nearest neighbor: gm = top8[:,1], idx = idx8[:,1].
  - host: dist^2 = n_t + n_idx - 2*gm from precomputed norms;
        loss = -mean(log(sqrt(dist^2) + 1e-8)) in f64.
"""

import numpy as np
import ml_dtypes

import concourse.bass as bass
import concourse.tile as tile
from concourse import bacc, mybir
from concourse import bass_utils

F32 = mybir.dt.float32
BF16 = mybir.dt.bfloat16
U32 = mybir.dt.uint32

OUT_NAMES = ("gm", "ix")

B, T, D = 8, 4096, 256
P = 128                  # partitions
M = T // P               # 32 m-tiles
KC = D // P              # 2 contraction chunks
EPS = 1e-8


def build_bass(num_devices=8):
    nc = bacc.Bacc("TRN2", target_bir_lowering=False, debug=False,
                   num_devices=num_devices)
    xT = nc.dram_tensor("xT", [KC, P, T], BF16, kind="ExternalInput")
    gm_out = nc.dram_tensor("gm", [P, M * 8], F32, kind="ExternalOutput")
    ix_out = nc.dram_tensor("ix", [P, M * 8], U32, kind="ExternalOutput")

    with tile.TileContext(nc) as tc:
        with (
            tc.tile_pool(name="const", bufs=1) as const_pool,
            tc.tile_pool(name="dots", bufs=3) as dots_pool,
            tc.tile_pool(name="psum", bufs=2, space="PSUM") as psum_pool,
            tc.tile_pool(name="res", bufs=1) as res_pool,
        ):
            # resident bf16 xT chunks
            xT_sb = [const_pool.tile([P, T], BF16, name=f"xT{c}", tag=f"xT{c}")
                     for c in range(KC)]
            for c in range(KC):
                nc.sync.dma_start(xT_sb[c][:], xT[c])

            gm_all = res_pool.tile([P, M * 8], F32, tag="gm")
            ix_all = res_pool.tile([P, M * 8], U32, tag="ix")

            for m in range(M):
                dots = dots_pool.tile([P, T], F32, tag="dots")
                for h in range(2):          # two psum halves of 4 n-blocks
                    ps = psum_pool.tile([P, 2048], F32, tag="ps")
                    for jj in range(4):
                        j = 4 * h + jj
                        for c in range(KC):
                            nc.tensor.matmul(
                                ps[:, jj * 512:(jj + 1) * 512],
                                lhsT=xT_sb[c][:, m * P:(m + 1) * P],
                                rhs=xT_sb[c][:, j * 512:(j + 1) * 512],
                                start=(c == 0), stop=(c == KC - 1))
                    for jj in range(4):
                        j = 4 * h + jj
                        nc.scalar.copy(dots[:, j * 512:(j + 1) * 512],
                                       ps[:, jj * 512:(jj + 1) * 512])

                # top-8 values + indices; diag self-dot is always top-1
                nc.vector.max(out=gm_all[:, 8 * m:8 * m + 8], in_=dots[:])
                nc.vector.max_index(out=ix_all[:, 8 * m:8 * m + 8],
                                    in_max=gm_all[:, 8 * m:8 * m + 8],
                                    in_values=dots[:])

            nc.sync.dma_start(gm_out[:], gm_all[:])
            nc.sync.dma_start(ix_out[:], ix_all[:])
    nc.compile()
    return nc


_CACHE = {}


def _built():
    if "nc" not in _CACHE:
        _CACHE["nc"] = build_bass(8)
    return _CACHE["nc"]


def make_in_maps(x):
    x = np.ascontiguousarray(np.asarray(x, dtype=np.float32))
    assert x.shape == (B, T, D)
    in_maps = []
    for b in range(B):
        xb = x[b]
        xTb = np.ascontiguousarray(xb.T).reshape(KC, P, T)
        in_maps.append({"xT": xTb.astype(ml_dtypes.bfloat16)})
    return in_maps


def postprocess(x, per_core):
    # per_core: list of (gm [128, 32*8] f32, ix [128, 32*8] u32).
    # Row t = 128*m + p; columns 8m..8m+7 hold the top-8 of tile m and
    # column 8m+0 is the diagonal self-dot, so the neighbor is 8m+1 —
    # unless the freak case where the diag is not top-1 (then use 8m+0).
    total = 0.0
    n = 0
    pp, mm = np.meshgrid(np.arange(P), np.arange(M), indexing="ij")
    t = (128 * mm + pp).reshape(-1)
    for b, (gm, ix) in enumerate(per_core):
        xb = np.asarray(x[b], dtype=np.float64)
        norms = np.einsum("td,td->t", xb, xb)
        g8 = gm.reshape(P, M, 8).astype(np.float64)
        i8 = ix.reshape(P, M, 8).astype(np.int64)
        top1_is_diag = i8[:, :, 0].reshape(-1) == t
        g = np.where(top1_is_diag, g8[:, :, 1].reshape(-1),
                     g8[:, :, 0].reshape(-1))
        i = np.where(top1_is_diag, i8[:, :, 1].reshape(-1),
                     i8[:, :, 0].reshape(-1))
        i = np.clip(i, 0, T - 1)
        d2 = norms[t] + norms[i] - 2.0 * g
        d2 = np.maximum(d2, 0.0)
        dist = np.sqrt(d2)
        total += np.log(dist + EPS).sum()
        n += dist.size
    return np.float32(-(total / n))


def kernel(student_output):
    nc = _built()
    in_maps = make_in_maps(student_output)
    res = bass_utils.run_bass_kernel_spmd(nc, in_maps, core_ids=list(range(B)))
    per_core = [(res.results[b]["gm"], res.results[b]["ix"]) for b in range(B)]
    return postprocess(student_output, per_core)


# revision 7
# speedup vs baseline: 8107.4482x; 1.0035x over previous
"""KoLeo-loss kernel for Trainium2 (Bass/Tile), data-parallel over batch on 8 cores.

Input : student_output [8, 4096, 256] fp32
Output: scalar fp32 loss = -mean(log(||x - x_nn + 1e-8||_2 + 1e-8))
        where x_nn[b,t] = x[b, argmax_s <x[b,t], x[b,s]> (diag excluded)].

Per-core plan (core b handles batch b):
  - PE: gram matrix dots = x @ x.T in 32 m-tiles of [128, 4096], bf16
        inputs (fp32 PSUM accumulation; bf16 matmul streams 1 col/cycle
        vs 4 for fp32).
  - ACT: PSUM -> SBUF fp32 copies (frees PSUM for the next half-tile).
  - DVE: InstMax top-8 + InstMaxIndex over each staged [128, 4096] row.
        The diagonal self-dot (~256) is always the row max (off-diag
        maxes are ~65 for this data), so column 1 of the top-8 is the
        nearest neighbor: gm = top8[:,1], idx = idx8[:,1].
  - host: dist^2 = n_t + n_idx - 2*gm from precomputed norms;
        loss = -mean(log(sqrt(dist^2) + 1e-8)) in f64.
"""

import numpy as np
import ml_dtypes

import concourse.bass as bass
import concourse.tile as tile
from concourse import bacc, mybir
from concourse import bass_utils

F32 = mybir.dt.float32
BF16 = mybir.dt.bfloat16
F16 = mybir.dt.float16
U32 = mybir.dt.uint32

OUT_NAMES = ("gm", "ix")

B, T, D = 8, 4096, 256
P = 128                  # partitions
M = T // P               # 32 m-tiles
KC = D // P              # 2 contraction chunks
EPS = 1e-8


def build_bass(num_devices=8):
    nc = bacc.Bacc("TRN2", target_bir_lowering=False, debug=False,
                   num_devices=num_devices)
    xT = nc.dram_tensor("xT", [KC, P, T], BF16, kind="ExternalInput")
    gm_out = nc.dram_tensor("gm", [P, M * 8], F16, kind="ExternalOutput")
    ix_out = nc.dram_tensor("ix", [P, M * 8], U32, kind="ExternalOutput")

    with tile.TileContext(nc) as tc:
        with (
            tc.tile_pool(name="const", bufs=1) as const_pool,
            tc.tile_pool(name="dots", bufs=3) as dots_pool,
            tc.tile_pool(name="psum", bufs=2, space="PSUM") as psum_pool,
            tc.tile_pool(name="res", bufs=1) as res_pool,
        ):
            # resident bf16 xT chunks
            xT_sb = [const_pool.tile([P, T], BF16, name=f"xT{c}", tag=f"xT{c}")
                     for c in range(KC)]
            for c in range(KC):
                nc.sync.dma_start(xT_sb[c][:], xT[c])

            gm_all = res_pool.tile([P, M * 8], F16, tag="gm")
            ix_all = res_pool.tile([P, M * 8], U32, tag="ix")

            for m in range(M):
                dots = dots_pool.tile([P, T], F16, tag="dots")
                for h in range(2):          # two psum halves of 4 n-blocks
                    ps = psum_pool.tile([P, 2048], F32, tag="ps")
                    for jj in range(4):
                        j = 4 * h + jj
                        for c in range(KC):
                            nc.tensor.matmul(
                                ps[:, jj * 512:(jj + 1) * 512],
                                lhsT=xT_sb[c][:, m * P:(m + 1) * P],
                                rhs=xT_sb[c][:, j * 512:(j + 1) * 512],
                                start=(c == 0), stop=(c == KC - 1))
                    for jj in range(4):
                        j = 4 * h + jj
                        nc.scalar.copy(dots[:, j * 512:(j + 1) * 512],
                                       ps[:, jj * 512:(jj + 1) * 512])

                # top-8 values + indices; diag self-dot is always top-1
                nc.vector.max(out=gm_all[:, 8 * m:8 * m + 8], in_=dots[:])
                nc.vector.max_index(out=ix_all[:, 8 * m:8 * m + 8],
                                    in_max=gm_all[:, 8 * m:8 * m + 8],
                                    in_values=dots[:])

            nc.sync.dma_start(gm_out[:], gm_all[:])
            nc.sync.dma_start(ix_out[:], ix_all[:])
    nc.compile()
    return nc


_CACHE = {}


def _built():
    if "nc" not in _CACHE:
        _CACHE["nc"] = build_bass(8)
    return _CACHE["nc"]


def make_in_maps(x):
    x = np.ascontiguousarray(np.asarray(x, dtype=np.float32))
    assert x.shape == (B, T, D)
    in_maps = []
    for b in range(B):
        xb = x[b]
        xTb = np.ascontiguousarray(xb.T).reshape(KC, P, T)
        in_maps.append({"xT": xTb.astype(ml_dtypes.bfloat16)})
    return in_maps


def postprocess(x, per_core):
    # per_core: list of (gm [128, 32*8] f32, ix [128, 32*8] u32).
    # Row t = 128*m + p; columns 8m..8m+7 hold the top-8 of tile m and
    # column 8m+0 is the diagonal self-dot, so the neighbor is 8m+1 —
    # unless the freak case where the diag is not top-1 (then use 8m+0).
    total = 0.0
    n = 0
    pp, mm = np.meshgrid(np.arange(P), np.arange(M), indexing="ij")
    t = (128 * mm + pp).reshape(-1)
    for b, (gm, ix) in enumerate(per_core):
        xb = np.asarray(x[b], dtype=np.float64)
        norms = np.einsum("td,td->t", xb, xb)
        g8 = gm.reshape(P, M, 8).astype(np.float64)
        i8 = ix.reshape(P, M, 8).astype(np.int64)
        top1_is_diag = i8[:, :, 0].reshape(-1) == t
        g = np.where(top1_is_diag, g8[:, :, 1].reshape(-1),
                     g8[:, :, 0].reshape(-1))
        i = np.where(top1_is_diag, i8[:, :, 1].reshape(-1),
                     i8[:, :, 0].reshape(-1))
        i = np.clip(i, 0, T - 1)
        d2 = norms[t] + norms[i] - 2.0 * g
        d2 = np.maximum(d2, 0.0)
        dist = np.sqrt(d2)
        total += np.log(dist + EPS).sum()
        n += dist.size
    return np.float32(-(total / n))


def kernel(student_output):
    nc = _built()
    in_maps = make_in_maps(student_output)
    res = bass_utils.run_bass_kernel_spmd(nc, in_maps, core_ids=list(range(B)))
    per_core = [(res.results[b]["gm"], res.results[b]["ix"]) for b in range(B)]
    return postprocess(student_output, per_core)
